# revision 50
# baseline (speedup 1.0000x reference)
"""Trainium2 Bass kernel for the phase-type log-prior problem.

reference(w, S, alpha) = sum_m log( alpha^T expm(w_m * S) s ),  s = -S @ 1

Since S is a fixed matrix and w_m are scalars, expm(w_m S) = V diag(exp(w_m d)) V^-1
with (d, V) the eigendecomposition of S (real eigenvalues for the lower-triangular
phase-type sub-generator this problem uses).  Hence

    density[m] = sum_i c_i * exp(d_i * w_m),   c = (alpha^T V) * (V^-1 s)

Device kernel (per core, raw Bass for minimal overhead): one DMA of the
shard [128, J] (plus the 8 biases ln|c_i| packed in the same row), 8 Exp
activations with per-term scale/bias on ScalarE, an fp32 add-chain on
VectorE that trails the activations (positive terms first, then negative,
one subtract), then Ln with per-partition accumulation on ScalarE, and one
DMA of the 128 partial sums back out.  Data-parallel over 8 cores; host
sums the 1024 partials in fp64 and removes the padding contribution.
"""

import os
import sys

import numpy as np

sys.path.insert(0, "/opt/trn_rl_repo")

import concourse.bass as bass  # noqa: E402
import concourse.mybir as mybir  # noqa: E402
from concourse.bass_utils import run_bass_kernel_spmd  # noqa: E402

N_CORES = 8
F32 = mybir.dt.float32
AF = mybir.ActivationFunctionType

_program_cache: dict = {}
_last_results = None


def _build_program(d: np.ndarray, logc: np.ndarray, n_neg: int, P: int, J: int):
    """Raw-Bass SPMD program, direct emission (no Block, no tail barrier).

    d, logc: per-term exp scale / bias ln|c_i|, NEGATIVE c terms first (so the
    negative add-chain finishes early and only the positive chain's last add
    plus the subtract trail the final exp).
    P, J: per-core tile layout [P partitions, J free]; shard size = P*J.
    Padding is handled on the host (pad value 1.0; its known log-density is
    subtracted from the total), so the device treats every element as real.
    The packed input row layout is: J w-values, n_terms biases, one 0.0
    (used as the Ln bias so no engine depends on the init-time const pool,
    which we strip below along with the init barrier).
    """
    n_terms = len(d)
    n_pos = n_terms - n_neg
    assert n_pos >= 1

    nc = bass.Bass()
    ncols = J + n_terms + 2     # w | biases | 0.0 (Ln bias) | 1.0 (sum lhsT)
    wb_in = nc.declare_dram_parameter("wb", [P, ncols], F32, isOutput=False)
    out = nc.declare_dram_parameter("partials", [1, 1], F32, isOutput=True)
    dram_scratch = nc.dram_tensor("warmup_scratch", [1, 1], F32)

    from contextlib import ExitStack
    with (
        nc.sbuf_tensor([P, ncols], F32) as WBt,
        nc.sbuf_tensor([P, n_terms * J], F32) as Ft,
        nc.sbuf_tensor([P, J], F32) as accp_t,
        nc.sbuf_tensor([P, J], F32) as accn_t,
        nc.sbuf_tensor([P, J], F32) as logd_t,
        nc.sbuf_tensor([P, 2], F32) as scratch_t,
        nc.psum_tensor([1, 1], F32) as psum_t,
        nc.semaphore("s_in") as s_in,
        nc.semaphore("s_act") as s_act,
        nc.semaphore("s_dve") as s_dve,
        nc.semaphore("s_pe") as s_pe,
        nc.semaphore("s_out") as s_out,
        ExitStack() as mm_ctx,
    ):
        WB = WBt[:]
        F = Ft[:]
        accp = accp_t[:]
        accn = accn_t[:]
        logd = logd_t[:]
        scratch = scratch_t[:]
        W = WB[:, 0:J]
        B = WB[:, J:J + n_terms]
        zbias = WB[:, J + n_terms:J + n_terms + 1]
        ones = WB[:, J + n_terms + 1:J + n_terms + 2]
        Fi = [F[:, i * J:(i + 1) * J] for i in range(n_terms)]
        part = scratch[:, 0:1]
        res_sb = scratch[0:1, 1:2]
        psum = psum_t[:]

        # --- Input DMA split in quarters across both HWDGE engines
        # (SP + Activation) so the pieces stream in parallel. ---
        Q = P // 4
        nc.sync.dma_start(WB[0:Q, :], wb_in[0:Q, :]).then_inc(s_in, 16)
        nc.sync.dma_start(WB[Q:2 * Q, :], wb_in[Q:2 * Q, :]).then_inc(s_in, 16)
        nc.scalar.dma_start(WB[2 * Q:3 * Q, :], wb_in[2 * Q:3 * Q, :]
                            ).then_inc(s_in, 16)
        nc.scalar.dma_start(WB[3 * Q:P, :], wb_in[3 * Q:P, :]).then_inc(s_in, 16)

        # --- Scalar: dummy exp hoists the act-table load over the DMA ---
        nc.scalar.activation(scratch[0:1, 1:2], WB[0:1, 0:1], AF.Exp,
                             bias=WB[0:1, 0:1], scale=1.0).then_inc(s_act, 1)
        nc.scalar.wait_ge(s_in, 64)
        for i in range(n_terms):
            nc.scalar.activation(
                Fi[i], W, AF.Exp, bias=B[:, i:i + 1], scale=float(d[i]),
            ).then_inc(s_act, 1)

        # --- DVE: add-chains trailing the exps (F_i ready at s_act>=i+2).
        # The last positive term F_{n-1} is kept OUT of the chains; the final
        # single op  dens = F_last +/- D  is the only DVE work trailing the
        # final exp. ---
        n_dve = 0

        def emit_chain(acc, base, count):
            nonlocal n_dve
            if count == 1:
                return Fi[base]
            nc.vector.wait_ge(s_act, base + 1 + 2)
            nc.vector.tensor_add(acc, Fi[base], Fi[base + 1]).then_inc(s_dve, 1)
            n_dve += 1
            for k in range(2, count):
                nc.vector.wait_ge(s_act, base + k + 2)
                nc.vector.tensor_add(acc, acc, Fi[base + k]).then_inc(s_dve, 1)
                n_dve += 1
            return acc

        # neg chain: accn = F0+..+F_{nn-1}; then ONE early type-switching
        # subtract accp = F_nn - accn; then pure ADDs accp += F_k.  The tail
        # after the final exp is a single ADD whose predecessor is also an
        # ADD (uop-table switches between TENSOR_TENSOR ALU ops cost a
        # ~0.5us pipe reconfig drain; keep them off the critical tail).
        dens = accp
        if n_terms == 1:
            dens = Fi[0]
        elif n_neg == 0:
            emit_chain(accp, 0, n_terms)
        else:
            neg_ap = Fi[0] if n_neg == 1 else emit_chain(accn, 0, n_neg)
            nc.vector.wait_ge(s_act, n_neg + 2)        # F_{nn} ready
            nc.vector.tensor_sub(accp, Fi[n_neg], neg_ap).then_inc(s_dve, 1)
            n_dve += 1
            for k in range(n_neg + 1, n_terms):
                nc.vector.wait_ge(s_act, k + 2)
                nc.vector.tensor_add(accp, accp, Fi[k]).then_inc(s_dve, 1)
                n_dve += 1
        n_dve_ops = n_dve

        # --- Scalar: Ln with per-partition accumulation ---
        if n_dve_ops:
            nc.scalar.wait_ge(s_dve, n_dve_ops)
        nc.scalar.activation(logd, dens, AF.Ln, bias=zbias, scale=1.0,
                             accum_out=part).then_inc(s_act, 1)

        # --- PE: reduce the 128 per-partition sums to one value.  This keeps
        # the output DMA a single 4-byte descriptor; a [128,1] DMA costs ~8us
        # in serial HWDGE descriptor processing. ---
        nc.tensor.wait_ge(s_act, n_terms + 2)
        nc.tensor.matmul(psum[0:1, 0:1], ones, part,
                         start=True, stop=True).then_inc(s_pe, 1)
        # PSUM -> SBUF copy and the 4-byte output DMA both on Scalar
        # (HWDGE-capable): program order replaces two cross-engine semaphore
        # hops, and Sync then reaches the exit barrier right after the input
        # DMAs, so the wave-1 barrier gate is Scalar's copy+DMA instead of
        # a longer Sync chain.  No completion wait on s_out: the walrus
        # postamble (sem-clear storm + exit barrier, ~6us) runs long past
        # the 4-byte transfer.
        nc.scalar.wait_ge(s_pe, 1)
        nc.scalar.copy(res_sb, psum[0:1, 0:1])
        nc.scalar.dma_start(out[:], res_sb, single_packet=True
                            ).then_inc(s_out, 16)

    _strip_init_overhead(nc)
    _hoist_dma_before_regmoves(nc)
    _split_multiwait(nc)
    return nc


def _hoist_dma_before_regmoves(nc):
    """Move each HWDGE engine's leading input DMAs in front of that engine's
    register-init moves (R8..R13 constants, unused by the DMA) so the
    transfers start a few hundred ns earlier."""
    for fn in nc.m.functions:
        for blk in fn.blocks:
            insts = blk.instructions
            for eng in (mybir.EngineType.SP, mybir.EngineType.Activation):
                first_mov = None
                dmas = []
                for idx, inst in enumerate(insts):
                    if inst.engine != eng:
                        continue
                    if isinstance(inst, mybir.InstRegisterMove):
                        if first_mov is None:
                            first_mov = idx
                        continue
                    if isinstance(inst, mybir.InstDMACopy):
                        dmas.append(idx)
                        continue
                    break  # engine's leading region ends at any other inst
                if first_mov is None or not dmas:
                    continue
                dmas = [i for i in dmas if i > first_mov]
                for k, idx in enumerate(dmas):
                    inst = insts.pop(idx)
                    insts.insert(first_mov + k, inst)


def _strip_init_overhead(nc):
    """Remove the Bass-init const-pool memsets and the init all-engine
    barrier.  Nothing in the program reads the const APs (the Ln bias comes
    from the packed input instead), so the barrier that orders them is dead
    weight (~1.5us of preamble)."""
    for fn in nc.m.functions:
        for blk in fn.blocks:
            kept = []
            for inst in blk.instructions:
                if isinstance(inst, mybir.InstMemset):
                    outs = inst.outs
                    name = ""
                    try:
                        name = outs[0].memorylocation.name
                    except Exception:
                        try:
                            name = outs[0].tensor.name
                        except Exception:
                            pass
                    if str(name).startswith("const-"):
                        continue
                if isinstance(inst, (mybir.InstDrain, mybir.InstEventSemaphore)):
                    si = getattr(inst, "sync_info", None)
                    refs = []
                    if si is not None:
                        refs = [w.ant_name for w in si.on_wait] + \
                               [u.ant_name for u in si.on_update]
                    if refs and all(str(r).startswith("barrier_") for r in refs):
                        continue
                kept.append(inst)
            blk.instructions[:] = kept


def _split_multiwait(nc, limit: int = 1):
    """walrus rejects instructions whose embedded sync-wait list exceeds the
    engine ISA struct's slots (1 for Activation, ~3 for Drain).  Hoist excess
    waits into standalone NoOps on the same engine just before the
    instruction."""
    k = 0
    for fn in nc.m.functions:
        for blk in fn.blocks:
            new = []
            for inst in blk.instructions:
                si = getattr(inst, "sync_info", None)
                if si is not None and si.on_wait and len(si.on_wait) > limit:
                    waits = list(si.on_wait)
                    for wchunk in waits[:-limit]:
                        k += 1
                        new.append(mybir.InstNoOp(
                            name=f"wsplit-{k}-{inst.name}",
                            sync_info=mybir.SyncInfo(on_wait=[wchunk],
                                                     on_update=[]),
                            bass_nofuse=True,
                            engine=inst.engine,
                        ))
                    inst.sync_info = mybir.SyncInfo(on_wait=waits[-limit:],
                                                    on_update=si.on_update)
                new.append(inst)
            blk.instructions[:] = new


def _ensure_ntff_hook() -> bool:
    """The agent image lacks ``antenv.axon_hooks``; synthesize it and register
    the ctypes NTFF profile hook so trace=True works under axon."""
    try:
        from antenv.axon_hooks import get_axon_ntff_profile_hook
        return get_axon_ntff_profile_hook() is not None
    except ImportError:
        pass
    try:
        import types

        import antenv
        from trn_agent_boot.trn_boot import _ntff_profile_via_ctypes

        mod = types.ModuleType("antenv.axon_hooks")
        holder = {"hook": None}
        mod.set_axon_ntff_profile_hook = lambda h: holder.__setitem__("hook", h)
        mod.get_axon_ntff_profile_hook = lambda: holder["hook"]
        sys.modules["antenv.axon_hooks"] = mod
        antenv.axon_hooks = mod
        hook = _ntff_profile_via_ctypes("/opt/axon/libaxon_pjrt.so")
        if hook is None:
            return False
        mod.set_axon_ntff_profile_hook(hook)
        return True
    except Exception as e:  # pragma: no cover - profiling is best-effort
        print(f"NTFF hook setup failed: {e}", file=sys.stderr)
        return False


def _spectral_coeffs(S: np.ndarray, alpha: np.ndarray):
    """c_i, d_i with density(w) = sum_i c_i exp(d_i w).  Returns None if the
    eigendecomposition is complex/ill-conditioned (not the case for the
    phase-type sub-generators this problem builds)."""
    S64 = S.astype(np.float64)
    s_vec = -S64.sum(axis=1)
    try:
        d, V = np.linalg.eig(S64)
        c = (alpha.astype(np.float64) @ V) * np.linalg.solve(V, s_vec)
    except np.linalg.LinAlgError:
        return None
    if np.abs(d.imag).max() > 1e-8 or np.abs(c.imag).max() > 1e-6 * max(
            1.0, np.abs(c.real).max()):
        return None
    if not (np.isfinite(c.real).all() and np.isfinite(d.real).all()):
        return None
    return c.real.copy(), d.real.copy()


def _host_fallback(w, S, alpha):
    """Exact host computation for inputs outside the real-spectral fast path
    (complex eigenvalues / defective S).  Batched scaling-and-squaring expm
    in fp64 via numpy."""
    S64 = S.astype(np.float64)
    s_vec = -S64.sum(axis=1)
    w64 = w.astype(np.float64)
    n = S64.shape[0]
    A = w64[:, None, None] * S64          # [M, n, n]
    nrm = np.abs(A).sum(axis=2).max(axis=1)
    k = np.maximum(0, np.ceil(np.log2(np.maximum(nrm, 1e-300))) + 1).astype(int)
    kmax = int(k.max()) if len(k) else 0
    A = A / (2.0 ** k)[:, None, None]
    # Pade(7) approximant
    b = [17297280., 8648640., 1995840., 277200., 25200., 1512., 56., 1.]
    I = np.eye(n)
    A2 = A @ A
    A4 = A2 @ A2
    A6 = A4 @ A2
    U = A @ (b[7] * A6 + b[5] * A4 + b[3] * A2 + b[1] * I)
    Vp = b[6] * A6 + b[4] * A4 + b[2] * A2 + b[0] * I
    E = np.linalg.solve(Vp - U, Vp + U)
    for j in range(kmax):
        sel = k > j
        E[sel] = E[sel] @ E[sel]
    dens = np.einsum("i,mij,j->m", alpha.astype(np.float64), E, s_vec)
    return np.float32(np.log(dens).sum())


def kernel(w: np.ndarray, S: np.ndarray, alpha: np.ndarray) -> np.ndarray:
    w = np.ascontiguousarray(np.asarray(w).reshape(-1), dtype=np.float32)
    S = np.asarray(S, dtype=np.float32)
    alpha = np.asarray(alpha, dtype=np.float32)

    cd = _spectral_coeffs(S, alpha)
    if cd is None:
        return _host_fallback(w, S, alpha)
    c, d = cd
    # Drop numerically-zero terms, order NEGATIVES first (their add-chain
    # then finishes early; see _build_program).
    keep = np.abs(c) > 1e-300
    c, d = c[keep], d[keep]
    order = np.argsort(c > 0, kind="stable")
    c, d = c[order], d[order]
    n_neg = int((c < 0).sum())
    logc = np.log(np.abs(c))

    M = w.size
    per = -(-M // N_CORES)          # ceil
    P = 128
    J = -(-per // P)                # ceil -> shard size P*J
    shard = P * J
    PAD_VAL = 1.0
    n_pad_total = N_CORES * shard - M
    n_terms = d.size
    shards = []
    for i in range(N_CORES):
        lo = min(i * per, M)
        hi = min((i + 1) * per, M)
        sh = np.empty((P, J + n_terms + 2), np.float32)
        wrow = np.empty(shard, np.float32)
        wrow[:hi - lo] = w[lo:hi]
        wrow[hi - lo:] = PAD_VAL
        sh[:, :J] = wrow.reshape(P, J)
        sh[:, J:J + n_terms] = logc.astype(np.float32)
        sh[:, J + n_terms] = 0.0
        sh[:, J + n_terms + 1] = 1.0
        shards.append(sh)

    key = (d.tobytes(), logc.tobytes(), n_neg, P, J)
    nc = _program_cache.get(key)
    if nc is None:
        nc = _build_program(d, logc, n_neg, P, J)
        _program_cache[key] = nc

    in_maps = [{"wb": shards[i]} for i in range(N_CORES)]
    trace = bool(os.environ.get("KERNEL_TRACE"))
    if trace:
        trace = _ensure_ntff_hook()
    res = run_bass_kernel_spmd(nc, in_maps, list(range(N_CORES)), trace=trace)
    global _last_results
    _last_results = res
    total = 0.0
    for r in res.results:
        total += r["partials"].astype(np.float64).sum()
    # Remove the host-known padding contribution log(density(PAD_VAL)).
    if n_pad_total:
        total -= n_pad_total * float(np.log(np.exp(d * PAD_VAL) @ c))
    return np.float32(total)


if __name__ == "__main__":
    z = np.load("/root/problem/inputs_cache.npz")
    out = kernel(z["w"], z["S"], z["alpha"])
    print("kernel output:", out)


# revision 52
# speedup vs baseline: 1.0001x; 1.0001x over previous
"""Trainium2 Bass kernel for the phase-type log-prior problem.

reference(w, S, alpha) = sum_m log( alpha^T expm(w_m * S) s ),  s = -S @ 1

Since S is a fixed matrix and w_m are scalars, expm(w_m S) = V diag(exp(w_m d)) V^-1
with (d, V) the eigendecomposition of S (real eigenvalues for the lower-triangular
phase-type sub-generator this problem uses).  Hence

    density[m] = sum_i c_i * exp(d_i * w_m),   c = (alpha^T V) * (V^-1 s)

Device kernel (per core, raw Bass for minimal overhead): one DMA of the
shard [128, J] (plus the 8 biases ln|c_i| packed in the same row), 8 Exp
activations with per-term scale/bias on ScalarE, an fp32 add-chain on
VectorE that trails the activations (positive terms first, then negative,
one subtract), then Ln with per-partition accumulation on ScalarE, and one
DMA of the 128 partial sums back out.  Data-parallel over 8 cores; host
sums the 1024 partials in fp64 and removes the padding contribution.
"""

import os
import sys

import numpy as np

sys.path.insert(0, "/opt/trn_rl_repo")

import concourse.bass as bass  # noqa: E402
import concourse.mybir as mybir  # noqa: E402
from concourse.bass_utils import run_bass_kernel_spmd  # noqa: E402

N_CORES = 8
F32 = mybir.dt.float32
AF = mybir.ActivationFunctionType

_program_cache: dict = {}
_last_results = None


def _build_program(d: np.ndarray, logc: np.ndarray, n_neg: int, P: int, J: int):
    """Raw-Bass SPMD program, direct emission (no Block, no tail barrier).

    d, logc: per-term exp scale / bias ln|c_i|, NEGATIVE c terms first (so the
    negative add-chain finishes early and only the positive chain's last add
    plus the subtract trail the final exp).
    P, J: per-core tile layout [P partitions, J free]; shard size = P*J.
    Padding is handled on the host (pad value 1.0; its known log-density is
    subtracted from the total), so the device treats every element as real.
    The packed input row layout is: J w-values, n_terms biases, one 0.0
    (used as the Ln bias so no engine depends on the init-time const pool,
    which we strip below along with the init barrier).
    """
    n_terms = len(d)
    n_pos = n_terms - n_neg
    assert n_pos >= 1

    nc = bass.Bass()
    ncols = J + n_terms + 2     # w | biases | 0.0 (Ln bias) | 1.0 (sum lhsT)
    wb_in = nc.declare_dram_parameter("wb", [P, ncols], F32, isOutput=False)
    out = nc.declare_dram_parameter("partials", [1, 1], F32, isOutput=True)

    with (
        nc.sbuf_tensor([P, ncols], F32) as WBt,
        nc.sbuf_tensor([P, n_terms * J], F32) as Ft,
        nc.sbuf_tensor([P, J], F32) as accp_t,
        nc.sbuf_tensor([P, J], F32) as accn_t,
        nc.sbuf_tensor([P, J], F32) as logd_t,
        nc.sbuf_tensor([P, 2], F32) as scratch_t,
        nc.psum_tensor([1, 1], F32) as psum_t,
        nc.semaphore("s_in") as s_in,
        nc.semaphore("s_act") as s_act,
        nc.semaphore("s_dve") as s_dve,
        nc.semaphore("s_pe") as s_pe,
        nc.semaphore("s_out") as s_out,
    ):
        WB = WBt[:]
        F = Ft[:]
        accp = accp_t[:]
        accn = accn_t[:]
        logd = logd_t[:]
        scratch = scratch_t[:]
        W = WB[:, 0:J]
        B = WB[:, J:J + n_terms]
        zbias = WB[:, J + n_terms:J + n_terms + 1]
        ones = WB[:, J + n_terms + 1:J + n_terms + 2]
        Fi = [F[:, i * J:(i + 1) * J] for i in range(n_terms)]
        part = scratch[:, 0:1]
        res_sb = scratch[0:1, 1:2]
        psum = psum_t[:]

        # --- Input DMA split in quarters across both HWDGE engines
        # (SP + Activation) so the pieces stream in parallel. ---
        Q = P // 4
        nc.sync.dma_start(WB[0:Q, :], wb_in[0:Q, :]).then_inc(s_in, 16)
        nc.sync.dma_start(WB[Q:2 * Q, :], wb_in[Q:2 * Q, :]).then_inc(s_in, 16)
        nc.scalar.dma_start(WB[2 * Q:3 * Q, :], wb_in[2 * Q:3 * Q, :]
                            ).then_inc(s_in, 16)
        nc.scalar.dma_start(WB[3 * Q:P, :], wb_in[3 * Q:P, :]).then_inc(s_in, 16)

        # --- Scalar: dummy exp hoists the act-table load over the DMA ---
        nc.scalar.activation(scratch[0:1, 1:2], WB[0:1, 0:1], AF.Exp,
                             bias=WB[0:1, 0:1], scale=1.0).then_inc(s_act, 1)
        nc.scalar.wait_ge(s_in, 64)
        for i in range(n_terms):
            nc.scalar.activation(
                Fi[i], W, AF.Exp, bias=B[:, i:i + 1], scale=float(d[i]),
            ).then_inc(s_act, 1)

        # --- DVE: add-chains trailing the exps (F_i ready at s_act>=i+2).
        # The last positive term F_{n-1} is kept OUT of the chains; the final
        # single op  dens = F_last +/- D  is the only DVE work trailing the
        # final exp. ---
        n_dve = 0

        def emit_chain(acc, base, count):
            nonlocal n_dve
            if count == 1:
                return Fi[base]
            nc.vector.wait_ge(s_act, base + 1 + 2)
            nc.vector.tensor_add(acc, Fi[base], Fi[base + 1]).then_inc(s_dve, 1)
            n_dve += 1
            for k in range(2, count):
                nc.vector.wait_ge(s_act, base + k + 2)
                nc.vector.tensor_add(acc, acc, Fi[base + k]).then_inc(s_dve, 1)
                n_dve += 1
            return acc

        # neg chain: accn = F0+..+F_{nn-1}; then ONE early type-switching
        # subtract accp = F_nn - accn; then pure ADDs accp += F_k.  The tail
        # after the final exp is a single ADD whose predecessor is also an
        # ADD (uop-table switches between TENSOR_TENSOR ALU ops cost a
        # ~0.5us pipe reconfig drain; keep them off the critical tail).
        dens = accp
        if n_terms == 1:
            dens = Fi[0]
        elif n_neg == 0:
            emit_chain(accp, 0, n_terms)
        else:
            neg_ap = Fi[0] if n_neg == 1 else emit_chain(accn, 0, n_neg)
            nc.vector.wait_ge(s_act, n_neg + 2)        # F_{nn} ready
            nc.vector.tensor_sub(accp, Fi[n_neg], neg_ap).then_inc(s_dve, 1)
            n_dve += 1
            for k in range(n_neg + 1, n_terms):
                nc.vector.wait_ge(s_act, k + 2)
                nc.vector.tensor_add(accp, accp, Fi[k]).then_inc(s_dve, 1)
                n_dve += 1
        n_dve_ops = n_dve

        # --- Scalar: Ln with per-partition accumulation ---
        if n_dve_ops:
            nc.scalar.wait_ge(s_dve, n_dve_ops)
        nc.scalar.activation(logd, dens, AF.Ln, bias=zbias, scale=1.0,
                             accum_out=part).then_inc(s_act, 1)

        # --- PE: reduce the 128 per-partition sums to one value.  This keeps
        # the output DMA a single 4-byte descriptor; a [128,1] DMA costs ~8us
        # in serial HWDGE descriptor processing. ---
        nc.tensor.wait_ge(s_act, n_terms + 2)
        nc.tensor.matmul(psum[0:1, 0:1], ones, part,
                         start=True, stop=True).then_inc(s_pe, 1)
        # PSUM -> SBUF copy and the 4-byte output DMA both on Scalar
        # (HWDGE-capable): program order replaces two cross-engine semaphore
        # hops, and Sync then reaches the exit barrier right after the input
        # DMAs, so the wave-1 barrier gate is Scalar's copy+DMA instead of
        # a longer Sync chain.  No completion wait on s_out: the walrus
        # postamble (sem-clear storm + exit barrier, ~6us) runs long past
        # the 4-byte transfer.
        nc.scalar.wait_ge(s_pe, 1)
        nc.scalar.copy(res_sb, psum[0:1, 0:1])
        nc.scalar.dma_start(out[:], res_sb, single_packet=True
                            ).then_inc(s_out, 16)

    _strip_init_overhead(nc)
    _hoist_dma_before_regmoves(nc)
    _split_multiwait(nc)
    return nc


def _hoist_dma_before_regmoves(nc):
    """Move each HWDGE engine's leading input DMAs in front of that engine's
    register-init moves (R8..R13 constants, unused by the DMA) so the
    transfers start a few hundred ns earlier."""
    for fn in nc.m.functions:
        for blk in fn.blocks:
            insts = blk.instructions
            for eng in (mybir.EngineType.SP, mybir.EngineType.Activation):
                first_mov = None
                dmas = []
                for idx, inst in enumerate(insts):
                    if inst.engine != eng:
                        continue
                    if isinstance(inst, mybir.InstRegisterMove):
                        if first_mov is None:
                            first_mov = idx
                        continue
                    if isinstance(inst, mybir.InstDMACopy):
                        dmas.append(idx)
                        continue
                    break  # engine's leading region ends at any other inst
                if first_mov is None or not dmas:
                    continue
                dmas = [i for i in dmas if i > first_mov]
                for k, idx in enumerate(dmas):
                    inst = insts.pop(idx)
                    insts.insert(first_mov + k, inst)


def _strip_init_overhead(nc):
    """Remove the Bass-init const-pool memsets and the init all-engine
    barrier.  Nothing in the program reads the const APs (the Ln bias comes
    from the packed input instead), so the barrier that orders them is dead
    weight (~1.5us of preamble)."""
    for fn in nc.m.functions:
        for blk in fn.blocks:
            kept = []
            for inst in blk.instructions:
                if isinstance(inst, mybir.InstMemset):
                    outs = inst.outs
                    name = ""
                    try:
                        name = outs[0].memorylocation.name
                    except Exception:
                        try:
                            name = outs[0].tensor.name
                        except Exception:
                            pass
                    if str(name).startswith("const-"):
                        continue
                if isinstance(inst, (mybir.InstDrain, mybir.InstEventSemaphore)):
                    si = getattr(inst, "sync_info", None)
                    refs = []
                    if si is not None:
                        refs = [w.ant_name for w in si.on_wait] + \
                               [u.ant_name for u in si.on_update]
                    if refs and all(str(r).startswith("barrier_") for r in refs):
                        continue
                kept.append(inst)
            blk.instructions[:] = kept


def _split_multiwait(nc, limit: int = 1):
    """walrus rejects instructions whose embedded sync-wait list exceeds the
    engine ISA struct's slots (1 for Activation, ~3 for Drain).  Hoist excess
    waits into standalone NoOps on the same engine just before the
    instruction."""
    k = 0
    for fn in nc.m.functions:
        for blk in fn.blocks:
            new = []
            for inst in blk.instructions:
                si = getattr(inst, "sync_info", None)
                if si is not None and si.on_wait and len(si.on_wait) > limit:
                    waits = list(si.on_wait)
                    for wchunk in waits[:-limit]:
                        k += 1
                        new.append(mybir.InstNoOp(
                            name=f"wsplit-{k}-{inst.name}",
                            sync_info=mybir.SyncInfo(on_wait=[wchunk],
                                                     on_update=[]),
                            bass_nofuse=True,
                            engine=inst.engine,
                        ))
                    inst.sync_info = mybir.SyncInfo(on_wait=waits[-limit:],
                                                    on_update=si.on_update)
                new.append(inst)
            blk.instructions[:] = new


def _ensure_ntff_hook() -> bool:
    """The agent image lacks ``antenv.axon_hooks``; synthesize it and register
    the ctypes NTFF profile hook so trace=True works under axon."""
    try:
        from antenv.axon_hooks import get_axon_ntff_profile_hook
        return get_axon_ntff_profile_hook() is not None
    except ImportError:
        pass
    try:
        import types

        import antenv
        from trn_agent_boot.trn_boot import _ntff_profile_via_ctypes

        mod = types.ModuleType("antenv.axon_hooks")
        holder = {"hook": None}
        mod.set_axon_ntff_profile_hook = lambda h: holder.__setitem__("hook", h)
        mod.get_axon_ntff_profile_hook = lambda: holder["hook"]
        sys.modules["antenv.axon_hooks"] = mod
        antenv.axon_hooks = mod
        hook = _ntff_profile_via_ctypes("/opt/axon/libaxon_pjrt.so")
        if hook is None:
            return False
        mod.set_axon_ntff_profile_hook(hook)
        return True
    except Exception as e:  # pragma: no cover - profiling is best-effort
        print(f"NTFF hook setup failed: {e}", file=sys.stderr)
        return False


def _spectral_coeffs(S: np.ndarray, alpha: np.ndarray):
    """c_i, d_i with density(w) = sum_i c_i exp(d_i w).  Returns None if the
    eigendecomposition is complex/ill-conditioned (not the case for the
    phase-type sub-generators this problem builds)."""
    S64 = S.astype(np.float64)
    s_vec = -S64.sum(axis=1)
    try:
        d, V = np.linalg.eig(S64)
        c = (alpha.astype(np.float64) @ V) * np.linalg.solve(V, s_vec)
    except np.linalg.LinAlgError:
        return None
    if np.abs(d.imag).max() > 1e-8 or np.abs(c.imag).max() > 1e-6 * max(
            1.0, np.abs(c.real).max()):
        return None
    if not (np.isfinite(c.real).all() and np.isfinite(d.real).all()):
        return None
    return c.real.copy(), d.real.copy()


def _host_fallback(w, S, alpha):
    """Exact host computation for inputs outside the real-spectral fast path
    (complex eigenvalues / defective S).  Batched scaling-and-squaring expm
    in fp64 via numpy."""
    S64 = S.astype(np.float64)
    s_vec = -S64.sum(axis=1)
    w64 = w.astype(np.float64)
    n = S64.shape[0]
    A = w64[:, None, None] * S64          # [M, n, n]
    nrm = np.abs(A).sum(axis=2).max(axis=1)
    k = np.maximum(0, np.ceil(np.log2(np.maximum(nrm, 1e-300))) + 1).astype(int)
    kmax = int(k.max()) if len(k) else 0
    A = A / (2.0 ** k)[:, None, None]
    # Pade(7) approximant
    b = [17297280., 8648640., 1995840., 277200., 25200., 1512., 56., 1.]
    I = np.eye(n)
    A2 = A @ A
    A4 = A2 @ A2
    A6 = A4 @ A2
    U = A @ (b[7] * A6 + b[5] * A4 + b[3] * A2 + b[1] * I)
    Vp = b[6] * A6 + b[4] * A4 + b[2] * A2 + b[0] * I
    E = np.linalg.solve(Vp - U, Vp + U)
    for j in range(kmax):
        sel = k > j
        E[sel] = E[sel] @ E[sel]
    dens = np.einsum("i,mij,j->m", alpha.astype(np.float64), E, s_vec)
    return np.float32(np.log(dens).sum())


def kernel(w: np.ndarray, S: np.ndarray, alpha: np.ndarray) -> np.ndarray:
    w = np.ascontiguousarray(np.asarray(w).reshape(-1), dtype=np.float32)
    S = np.asarray(S, dtype=np.float32)
    alpha = np.asarray(alpha, dtype=np.float32)

    cd = _spectral_coeffs(S, alpha)
    if cd is None:
        return _host_fallback(w, S, alpha)
    c, d = cd
    # Drop numerically-zero terms, order NEGATIVES first (their add-chain
    # then finishes early; see _build_program).
    keep = np.abs(c) > 1e-300
    c, d = c[keep], d[keep]
    order = np.argsort(c > 0, kind="stable")
    c, d = c[order], d[order]
    n_neg = int((c < 0).sum())
    logc = np.log(np.abs(c))

    M = w.size
    per = -(-M // N_CORES)          # ceil
    P = 128
    J = -(-per // P)                # ceil -> shard size P*J
    shard = P * J
    PAD_VAL = 1.0
    n_pad_total = N_CORES * shard - M
    n_terms = d.size
    shards = []
    for i in range(N_CORES):
        lo = min(i * per, M)
        hi = min((i + 1) * per, M)
        sh = np.empty((P, J + n_terms + 2), np.float32)
        wrow = np.empty(shard, np.float32)
        wrow[:hi - lo] = w[lo:hi]
        wrow[hi - lo:] = PAD_VAL
        sh[:, :J] = wrow.reshape(P, J)
        sh[:, J:J + n_terms] = logc.astype(np.float32)
        sh[:, J + n_terms] = 0.0
        sh[:, J + n_terms + 1] = 1.0
        shards.append(sh)

    key = (d.tobytes(), logc.tobytes(), n_neg, P, J)
    nc = _program_cache.get(key)
    if nc is None:
        nc = _build_program(d, logc, n_neg, P, J)
        _program_cache[key] = nc

    in_maps = [{"wb": shards[i]} for i in range(N_CORES)]
    trace = bool(os.environ.get("KERNEL_TRACE"))
    if trace:
        trace = _ensure_ntff_hook()
    res = run_bass_kernel_spmd(nc, in_maps, list(range(N_CORES)), trace=trace)
    global _last_results
    _last_results = res
    total = 0.0
    for r in res.results:
        total += r["partials"].astype(np.float64).sum()
    # Remove the host-known padding contribution log(density(PAD_VAL)).
    if n_pad_total:
        total -= n_pad_total * float(np.log(np.exp(d * PAD_VAL) @ c))
    return np.float32(total)


if __name__ == "__main__":
    z = np.load("/root/problem/inputs_cache.npz")
    out = kernel(z["w"], z["S"], z["alpha"])
    print("kernel output:", out)
